# revision 29
# baseline (speedup 1.0000x reference)
"""MipRayMarcher2 volume-rendering kernel for Trainium2 (Bass/Tile), SPMD over 8 cores.

Reference semantics (see problem): per ray of S=48 samples,
  deltas    = depths[1:] - depths[:-1]                      # 47
  mids      = midpoints of colors/densities/depths/dinos    # 47
  dens_mid  = softplus(densities_mid - 1)
  alpha     = 1 - exp(-dens_mid * deltas)
  T         = cumprod([1, 1-alpha+1e-10])[:-1]              # transmittance
  weights   = alpha * T
  composite_rgb  = sum_s w * colors_mid   -> *2-1
  composite_dino = sum_s w * dinos_mid + 1 - wt -> *2-1
  composite_depth= sum_s w * depths_mid / wt, nan->inf, clip to [min,max] depths
  var_depth  = mean over rays of sum_s w*(depths_mid-mean)^2/(wt+1e-6)
  bg_lambda  = T[-1]

Key identity used on-device: for any per-sample channel x,
  sum_s w[s] * 0.5*(x[s] + x[s+1]) = 0.5 * sum_j wmod[j] * x[j]
with wmod[0]=w[0], wmod[j]=w[j-1]+w[j] (1<=j<=46), wmod[47]=w[46].
This avoids materializing the midpoints of the two big tensors.

Layout: rays on the 128 SBUF partitions; the whole per-ray "small" pipeline
(densities/depths -> weights/transmittance/scalars) is computed BATCHED for
all 32 ray-tiles in single wide instructions (DVE instruction overhead is
~150-270ns; 47-wide ops waste 60% of the engine). The per-tile loop only does
the two big weighted sums (colors [48x32], dinos [48x64] per ray).

The batched transmittance cumprod uses one tensor_tensor_scan over the
flattened [tile, sample] axis with a segmented-reset trick:
  state = data0*state + data1,  data0 = 0 at segment starts (else 1-alpha),
  data1 = 1 at segment starts (else 0)  ==> state resets to 1 per segment.

Sharding: batch*rays (2*16384 = 32768) flattened and split evenly across the
8 NeuronCores (4096 rays/core); every op is per-ray, so no collectives.
Global depth min/max, the tiny per-ray division for composite_depth, and the
final scalar mean for var_depth are done on host.
"""

import sys

import numpy as np

for _p in ("/opt/trn_rl_repo",):
    if _p not in sys.path:
        sys.path.insert(0, _p)

import concourse.bacc as bacc
import concourse.bass as bass  # noqa: F401
import concourse.tile as tile
from concourse import mybir
from concourse.bass_utils import run_bass_kernel_spmd

F32 = mybir.dt.float32
AF = mybir.ActivationFunctionType
OP = mybir.AluOpType

B, R, S, C, CD = 2, 16384, 48, 32, 64
N_CORES = 8
N_RAYS = B * R  # 32768
RAYS_PER_CORE = N_RAYS // N_CORES  # 4096
P = 128

# outpack column layout (per-tile composites)
O_RGB = 0            # 32 cols: sum_j wmod*colors - 1            (== rgb*2-1)
O_DINO = 32          # 64 cols: sum_j wmod*dinos + 1 - 2*wt      (== dino*2-1)
NPACK = 96

# scal column layout (per-ray scalars, batched)
SC_WT = 0            # weight_total
SC_BG = 1            # bg_lambda
SC_DSR = 2           # 2 * sum_s w*depths_mid   (host halves)
SC_VNR = 3           # 4 * sum_s w*(depths_mid-mean)^2  (host quarters)
NSCAL = 4


def build_nc(
    n_rays: int = RAYS_PER_CORE,
    dinos_mult_on_gpsimd: bool = True,
    repeats: int = 1,
    big_bufs: int = 4,
    small_bufs: int = 3,
    group: int = 1,
    bchunk: int = 32,
):
    """Build the per-core Bass module. Same program on all 8 cores.

    group: ray-tiles handled per big-DMA/compute instruction.
    bchunk: ray-tiles per batched small-pipeline chunk (chunking lets the
    per-group loop start before the whole batched phase finishes).
    repeats > 1 re-runs the whole pass (timing: differencing repeat counts
    cancels dispatch + one-time costs)."""
    assert n_rays % P == 0
    ntiles = n_rays // P
    group = min(group, ntiles)
    bchunk = min(bchunk, ntiles)
    assert ntiles % bchunk == 0 and bchunk % group == 0
    # Bacc (not plain Bass): its compile() pipeline runs
    # generate_event_semaphores(), which splits multi-sem waits into
    # standalone EventSemaphore instructions — TRN2 allows only 1 wait per
    # instruction and walrus codegen hard-errors otherwise.
    nc = bacc.Bacc()

    colors_d = nc.dram_tensor("colors", [n_rays, S * C], F32, kind="ExternalInput")
    dens_d = nc.dram_tensor("densities", [n_rays, S], F32, kind="ExternalInput")
    dep_d = nc.dram_tensor("depths", [n_rays, S], F32, kind="ExternalInput")
    dinos_d = nc.dram_tensor("dinos", [n_rays, S * CD], F32, kind="ExternalInput")
    out_d = nc.dram_tensor("outpack", [n_rays, NPACK], F32, kind="ExternalOutput")
    w_d = nc.dram_tensor("weights", [n_rays, S - 1], F32, kind="ExternalOutput")
    scal_d = nc.dram_tensor("scal", [n_rays, NSCAL], F32, kind="ExternalOutput")

    T = ntiles
    with tile.TileContext(nc) as tc:
        with (
            tc.tile_pool(name="singles", bufs=1) as singles,
            tc.tile_pool(name="batch", bufs=2) as bpool,
            tc.tile_pool(name="cpool", bufs=big_bufs) as cpool,
            tc.tile_pool(name="dpool", bufs=big_bufs) as dpool,
            tc.tile_pool(name="small", bufs=small_bufs) as small,
            tc.tile_pool(name="opool", bufs=small_bufs) as opool,
        ):
            bias_m1 = singles.tile([P, 1], F32)
            nc.vector.memset(bias_m1, -1.0)
            bias_eps = singles.tile([P, 1], F32)
            nc.vector.memset(bias_eps, 1e-10)

            dep_all = singles.tile([P, T, S], F32)
            den_all = singles.tile([P, T, S], F32)
            nc.sync.dma_start(
                out=dep_all, in_=dep_d.rearrange("(t p) s -> p t s", p=P)
            )
            nc.sync.dma_start(
                out=den_all, in_=dens_d.rearrange("(t p) s -> p t s", p=P)
            )

            # mask for the segmented cumprod: 1.0 at segment starts, else 0
            mask1 = singles.tile([P, bchunk, S], F32)
            nc.vector.memset(mask1, 0.0)
            nc.gpsimd.memset(mask1[:, :, 0:1], 1.0)

            nchunks = ntiles // bchunk
            # wmod / b2 per chunk live for the whole pass
            wmods = [
                singles.tile([P, bchunk, S], F32, name=f"wmod{j}", tag=f"wmod{j}")
                for j in range(nchunks)
            ]
            b2s = [
                singles.tile([P, bchunk], F32, name=f"b2_{j}", tag=f"b2_{j}")
                for j in range(nchunks)
            ]

            for _rep in range(repeats):
                _emit_pass(
                    nc, ntiles, group, bchunk, dinos_mult_on_gpsimd,
                    bpool, cpool, dpool, small, opool,
                    bias_m1, bias_eps, dep_all, den_all, mask1, wmods, b2s,
                    colors_d, dinos_d, out_d, w_d, scal_d,
                )

    nc.finalize()
    return nc


def _emit_pass(
    nc, ntiles, group, bchunk, dinos_mult_on_gpsimd,
    bpool, cpool, dpool, small, opool,
    bias_m1, bias_eps, dep_all, den_all, mask1, wmods, b2s,
    colors_d, dinos_d, out_d, w_d, scal_d,
):
    Tc = bchunk
    AX = mybir.AxisListType.X

    # ======== batched per-ray pipeline, one chunk of tiles at a time ======
    for j in range(ntiles // bchunk):
        j0 = j * bchunk
        dep = dep_all[:, j0 : j0 + Tc, :]
        den = den_all[:, j0 : j0 + Tc, :]

        deltas = bpool.tile([P, Tc, S - 1], F32)
        nc.vector.tensor_sub(deltas, dep[:, :, 1:S], dep[:, :, 0 : S - 1])
        nsum = bpool.tile([P, Tc, S - 1], F32)
        nc.vector.tensor_add(nsum, den[:, :, 1:S], den[:, :, 0 : S - 1])
        # softplus(0.5*(n0+n1) - 1) = ln(1 + exp(0.5*nsum - 1)); in-place
        nc.scalar.activation(nsum, nsum, AF.Exp, bias=bias_m1, scale=0.5)
        nc.scalar.activation(nsum, nsum, AF.Ln, bias=1.0, scale=1.0)
        # dd = softplus * deltas (in-place into deltas)
        nc.vector.tensor_mul(deltas, deltas, nsum)
        # e2 = exp(-dd) (in-place)
        nc.scalar.activation(deltas, deltas, AF.Exp, scale=-1.0)
        e2 = deltas
        # ash: 0 at col0 of each segment, am1 = e2 + 1e-10 elsewhere
        ash = bpool.tile([P, Tc, S], F32)
        nc.vector.memset(ash[:, :, 0:1], 0.0)
        nc.scalar.activation(
            ash[:, :, 1:S], e2, AF.Identity, bias=bias_eps, scale=1.0
        )
        # alpha = 1 - e2 (in-place over e2)
        nc.scalar.activation(e2, e2, AF.Copy, scale=-1.0, bias=1.0)
        alpha = e2
        # segmented cumprod: state = ash*state + mask1 -> resets to 1 at
        # segment starts; t48[:, t, k] = prod_{l<k} am1[l]
        t48 = bpool.tile([P, Tc, S], F32)
        nc.vector.tensor_tensor_scan(
            t48.rearrange("p t s -> p (t s)"),
            ash.rearrange("p t s -> p (t s)"),
            mask1.rearrange("p t s -> p (t s)"),
            0.0,
            OP.mult,
            OP.add,
        )
        # weights = alpha * T ; wt = sum(weights)
        w_all = bpool.tile([P, Tc, S - 1], F32)
        nc.vector.tensor_mul(w_all, alpha, t48[:, :, 0 : S - 1])
        scal = bpool.tile([P, Tc, NSCAL], F32)
        nc.vector.tensor_reduce(scal[:, :, SC_WT], w_all, axis=AX, op=OP.add)
        nc.scalar.copy(scal[:, :, SC_BG], t48[:, :, S - 2])
        # wmod (un-halved): [w0, w0+w1, ..., w45+w46, w46]
        wmod = wmods[j]
        nc.scalar.copy(wmod[:, :, 0], w_all[:, :, 0])
        nc.scalar.copy(wmod[:, :, S - 1], w_all[:, :, S - 2])
        nc.vector.tensor_add(
            wmod[:, :, 1 : S - 1],
            w_all[:, :, 0 : S - 2],
            w_all[:, :, 1 : S - 1],
        )
        # dsr = sum_l wmod*depths = 2*sum_s w*depths_mid (host halves)
        dscr = bpool.tile([P, Tc, S], F32)
        nc.vector.tensor_mul(dscr, wmod, dep)
        nc.vector.tensor_reduce(scal[:, :, SC_DSR], dscr, axis=AX, op=OP.add)
        # vnr = sum_l w*(dmr - dsr)^2 = 4*(var numerator) (host quarters)
        dmr = bpool.tile([P, Tc, S - 1], F32)
        nc.vector.tensor_add(dmr, dep[:, :, 1:S], dep[:, :, 0 : S - 1])
        nc.vector.tensor_sub(
            dmr,
            dmr,
            scal[:, :, SC_DSR : SC_DSR + 1].broadcast_to([P, Tc, S - 1]),
        )
        nc.scalar.activation(dmr, dmr, AF.Square)
        nc.vector.tensor_mul(dmr, dmr, w_all)
        nc.vector.tensor_reduce(scal[:, :, SC_VNR], dmr, axis=AX, op=OP.add)
        # b2 = 1 - 2*wt (per-ray bias for the dino composite)
        nc.scalar.activation(
            b2s[j], scal[:, :, SC_WT], AF.Copy, scale=-2.0, bias=1.0
        )
        r0 = j0 * P
        r1 = (j0 + Tc) * P
        nc.sync.dma_start(
            out=w_d[r0:r1, :].rearrange("(t p) s -> p t s", p=P), in_=w_all
        )
        nc.sync.dma_start(
            out=scal_d[r0:r1, :].rearrange("(t p) k -> p t k", p=P), in_=scal
        )

    # ======== grouped big weighted sums ========
    G = group
    for g in range(ntiles // G):
        t0 = g * G
        r0 = t0 * P
        j = t0 // bchunk
        lt = t0 - j * bchunk  # local tile index within chunk

        ctile = cpool.tile([P, G, S, C], F32)
        nc.sync.dma_start(
            out=ctile,
            in_=colors_d[r0 : r0 + G * P, :].rearrange(
                "(u p) (s c) -> p u s c", p=P, c=C
            ),
        )
        dtile = dpool.tile([P, G, S, CD], F32)
        nc.sync.dma_start(
            out=dtile,
            in_=dinos_d[r0 : r0 + G * P, :].rearrange(
                "(u p) (s c) -> p u s c", p=P, c=CD
            ),
        )

        wm = wmods[j][:, lt : lt + G, :]
        nc.vector.tensor_mul(
            ctile, ctile, wm[:, :, :, None].broadcast_to([P, G, S, C])
        )
        eng = nc.gpsimd if dinos_mult_on_gpsimd else nc.vector
        eng.tensor_mul(
            dtile, dtile, wm[:, :, :, None].broadcast_to([P, G, S, CD])
        )

        outp = opool.tile([P, G, NPACK], F32)
        rgbs = small.tile([P, G, C], F32)
        nc.vector.tensor_reduce(
            rgbs,
            ctile.rearrange("p u s c -> p u c s"),
            axis=mybir.AxisListType.X,
            op=OP.add,
        )
        dins = small.tile([P, G, CD], F32)
        nc.vector.tensor_reduce(
            dins,
            dtile.rearrange("p u s c -> p u c s"),
            axis=mybir.AxisListType.X,
            op=OP.add,
        )
        # rgb_out = 2*(0.5*rgbs) - 1 = rgbs - 1
        nc.scalar.activation(
            outp[:, :, O_RGB : O_RGB + C], rgbs, AF.Copy, scale=1.0, bias=-1.0
        )
        # dino_out = 2*(0.5*dins + 1 - wt) - 1 = dins + (1 - 2*wt)
        for u in range(G):
            nc.scalar.activation(
                outp[:, u, O_DINO : O_DINO + CD],
                dins[:, u, :],
                AF.Identity,
                scale=1.0,
                bias=b2s[j][:, lt + u : lt + u + 1],
            )
        nc.sync.dma_start(
            out=out_d[r0 : r0 + G * P, :].rearrange("(u p) k -> p u k", p=P),
            in_=outp,
        )


_CACHE: dict = {}
BUILD_KW: dict = {}


def _get_nc():
    if "nc" not in _CACHE:
        _CACHE["nc"] = build_nc(RAYS_PER_CORE, **BUILD_KW)
    return _CACHE["nc"]


def run_device(in_maps, trace=False, **kw):
    nc = _get_nc()
    return run_bass_kernel_spmd(nc, in_maps, list(range(N_CORES)), trace=trace, **kw)


def make_in_maps(colors, densities, depths, dinos):
    c2 = np.ascontiguousarray(np.asarray(colors, np.float32).reshape(N_RAYS, S * C))
    n2 = np.ascontiguousarray(np.asarray(densities, np.float32).reshape(N_RAYS, S))
    d2 = np.ascontiguousarray(np.asarray(depths, np.float32).reshape(N_RAYS, S))
    di2 = np.ascontiguousarray(np.asarray(dinos, np.float32).reshape(N_RAYS, S * CD))
    per = RAYS_PER_CORE
    return [
        {
            "colors": c2[i * per : (i + 1) * per],
            "densities": n2[i * per : (i + 1) * per],
            "depths": d2[i * per : (i + 1) * per],
            "dinos": di2[i * per : (i + 1) * per],
        }
        for i in range(N_CORES)
    ]


def postprocess(packs, w_out, scal, depths):
    """packs [N_RAYS, NPACK], w_out [N_RAYS, 47], scal [N_RAYS, 4] fp32."""
    rgb = packs[:, O_RGB : O_RGB + C].reshape(B, R, C)
    dino = packs[:, O_DINO : O_DINO + CD].reshape(B, R, CD)
    weights = w_out.reshape(B, R, S - 1, 1)
    wt = scal[:, SC_WT].reshape(B, R, 1)
    bg = scal[:, SC_BG].reshape(B, R, 1)
    dsum = (scal[:, SC_DSR] * np.float32(0.5)).reshape(B, R, 1)
    vnum = scal[:, SC_VNR] * np.float32(0.25)

    d = np.asarray(depths, np.float32)
    with np.errstate(divide="ignore", invalid="ignore"):
        cdep = dsum / wt
    cdep = np.where(np.isnan(cdep), np.float32(np.inf), cdep)
    cdep = np.clip(cdep, d.min(), d.max()).astype(np.float32)

    var_depth = np.float32(np.mean(vnum / (scal[:, SC_WT] + np.float32(1e-6))))

    return (rgb, cdep, weights, bg, dino, wt, var_depth)


def kernel(colors, densities, depths, dinos):
    in_maps = make_in_maps(colors, densities, depths, dinos)
    res = run_device(in_maps, trace=False)
    packs = np.concatenate([r["outpack"] for r in res.results], axis=0)
    w_out = np.concatenate([r["weights"] for r in res.results], axis=0)
    scal = np.concatenate([r["scal"] for r in res.results], axis=0)
    return postprocess(packs, w_out, scal, depths)


def _timed_device_run(nc, in_maps, reps):
    """Min wall time of one jitted NEFF execution with device-resident
    inputs (donated output buffers rotated between calls)."""
    import time

    import jax
    from jax.experimental.shard_map import shard_map
    from jax.sharding import Mesh, NamedSharding, PartitionSpec

    from concourse import bass2jax as b2j
    from concourse import mybir as _mb

    b2j.install_neuronx_cc_hook()

    part_name = nc.partition_id_tensor.name if nc.partition_id_tensor else None
    in_names, out_names, out_avals = [], [], []
    for alloc in nc.m.functions[0].allocations:
        if not isinstance(alloc, _mb.MemoryLocationSet):
            continue
        name = alloc.memorylocations[0].name
        if alloc.kind == "ExternalInput":
            if name != part_name:
                in_names.append(name)
        elif alloc.kind == "ExternalOutput":
            out_names.append(name)
            out_avals.append(
                jax.core.ShapedArray(
                    tuple(alloc.tensor_shape), _mb.dt.np(alloc.dtype)
                )
            )
    n_params = len(in_names)
    all_names = tuple(in_names + out_names)
    if part_name is not None:
        all_names = all_names + (part_name,)

    def _body(*args):
        ins = list(args[:n_params])
        bufs = list(args[n_params:])
        extra = [b2j.partition_id_tensor()] if part_name is not None else []
        outs = b2j._bass_exec_p.bind(
            *ins,
            *bufs,
            *extra,
            out_avals=tuple(out_avals),
            in_names=all_names,
            out_names=tuple(out_names),
            lowering_input_output_aliases=(),
            sim_require_finite=False,
            sim_require_nnan=False,
            nc=nc,
        )
        return tuple(outs)

    devices = jax.devices()[: N_CORES]
    mesh = Mesh(np.asarray(devices), ("core",))
    spec = NamedSharding(mesh, PartitionSpec("core"))
    nin = n_params + len(out_avals)

    concat_in = [
        np.concatenate([np.asarray(m[name]) for m in in_maps], axis=0)
        for name in in_names
    ]
    concat_zeros = [
        np.zeros((N_CORES * a.shape[0], *a.shape[1:]), a.dtype) for a in out_avals
    ]
    dev_in = [jax.device_put(a, spec) for a in concat_in]
    bufs = [jax.device_put(a, spec) for a in concat_zeros]

    f = jax.jit(
        shard_map(
            _body,
            mesh=mesh,
            in_specs=(PartitionSpec("core"),) * nin,
            out_specs=(PartitionSpec("core"),) * len(out_avals),
            check_rep=False,
        ),
        donate_argnums=tuple(range(n_params, nin)),
    )
    outs = f(*dev_in, *bufs)
    jax.block_until_ready(outs)
    bufs = list(outs)
    best = float("inf")
    for _ in range(reps):
        import time as _t

        t0 = _t.perf_counter()
        outs = f(*dev_in, *bufs)
        jax.block_until_ready(outs)
        best = min(best, _t.perf_counter() - t0)
        bufs = list(outs)
    return best


def measure_exec_ns(in_maps, iters=12, hi_repeats=17, **build_kw):
    """Estimate single-pass HW exec time by differencing NEFF executions of
    the per-tile loop repeated `hi_repeats` times vs once (cancels dispatch
    RTT + one-time costs). Interleaves the two configs to cancel drift.
    Returns (exec_ns, t_rep1_s, t_repN_s)."""
    kw = dict(BUILD_KW)
    kw.update(build_kw)
    nc1 = build_nc(RAYS_PER_CORE, repeats=1, **kw)
    ncN = build_nc(RAYS_PER_CORE, repeats=hi_repeats, **kw)
    t1 = min(_timed_device_run(nc1, in_maps, iters // 3) for _ in range(3))
    tN = min(_timed_device_run(ncN, in_maps, iters // 3) for _ in range(3))
    return (tN - t1) / (hi_repeats - 1) * 1e9, t1, tN
